# revision 1
# baseline (speedup 1.0000x reference)
"""Trainium2 Bass kernel for nn_ChunkedQuantHead.

Computation (see reference):
  xc   = x.reshape(B, 16, 256)
  acts = mean(|xc|, axis=(0, 2))           # global per-chunk stat
  top4 = top_k(acts, 4)                    # global chunk routing
  routed = einsum('bkc,koc->bo', xc[:, top4], expert_w[top4]) + expert_b
  w_eff  = quant_w if max(acts) > 0.5 else sign(quant_w)*mean|quant_w|
  out    = routed @ w_eff.T + quant_b

Strategy (8 cores, data-parallel over batch):
  - Each core streams its 2048x4096 f32 shard from HBM ONCE (bf16 cast
    during the SWDGE DMA).  In the same pass it computes per-chunk
    |x| partial sums (DVE fused abs+reduce) AND projects ALL 16 chunks
    (y_all[b, c, o]) via PE (transpose blocks + bf16 matmuls) -- so no
    second pass over x is ever needed.
  - A 64-byte AllReduce combines the chunk stats; top-4 selection is
    done with masks (sum of top-4 needs no ordering), so there is no
    dynamic control flow: routed = sum_c mask_c * y_all[:, c, :].
  - The quantized head runs on-chip: w_eff = w_bin + cond*(qw - w_bin),
    folded into a tiny [11,10] matmul (bias row included).
"""

import numpy as np

import concourse.bacc as bacc
import concourse.tile as tile
import concourse.mybir as mybir
from concourse.bass_utils import run_bass_kernel_spmd
from concourse.masks import make_identity

F32 = mybir.dt.float32
BF16 = mybir.dt.bfloat16
AX = mybir.AxisListType
OP = mybir.AluOpType

N_CORES = 8
B, F = 16384, 4096
CHUNKS, CHUNK, OUT = 16, 256, 10
TOPK = 4
THRESH = 0.5
BC = B // N_CORES            # 2048 rows per core
P = 128
TILES = BC // P              # 16 tiles of 128 rows
KH = F // P                  # 32 128-feature half-chunk blocks
SUM_THRESH = THRESH * B * CHUNK  # compare sum(|x|) against this (scale folded)
BIG_NEG = -1.0e30

_CACHE = {}


def _build():
    nc = bacc.Bacc(
        "TRN2",
        target_bir_lowering=False,
        debug=False,
        num_devices=N_CORES,
    )

    x_d = nc.dram_tensor("x", [BC, F], F32, kind="ExternalInput")
    # expert weights pre-arranged host-side: w_sb[p, h*160 + c*10 + o]
    #   = expert_w[c, o, h*128 + p]   (bf16)
    w_d = nc.dram_tensor("w_sb", [P, 2 * CHUNKS * OUT], BF16, kind="ExternalInput")
    # head weights, transposed layouts prepared host-side
    dT_d = nc.dram_tensor("dT", [OUT, OUT], F32, kind="ExternalInput")       # qw.T - w_bin.T
    wbinT_d = nc.dram_tensor("wbinT", [OUT, OUT], F32, kind="ExternalInput")  # sign(qw.T)*mean|qw|
    eb_d = nc.dram_tensor("eb", [1, OUT], F32, kind="ExternalInput")
    qb_d = nc.dram_tensor("qb", [1, OUT], F32, kind="ExternalInput")
    out_d = nc.dram_tensor("out", [BC, OUT], F32, kind="ExternalOutput")

    with tile.TileContext(nc) as tc:
        with (
            tc.tile_pool(name="const", bufs=1) as constp,
            tc.tile_pool(name="persist", bufs=1) as perp,
            tc.tile_pool(name="xb", bufs=3) as xbp,
            tc.tile_pool(name="xt", bufs=2) as xtp,
            tc.tile_pool(name="tail", bufs=2) as tailp,
            tc.tile_pool(name="ps_tr", bufs=3, space="PSUM") as pstr,
            tc.tile_pool(name="ps_y", bufs=2, space="PSUM") as psy,
            tc.tile_pool(name="ps_misc", bufs=2, space="PSUM") as psm,
            tc.tile_pool(name="dram", bufs=1, space="DRAM") as dramp,
        ):
            # ---- constants ----
            id_bf = constp.tile([P, P], BF16)
            make_identity(nc, id_bf)
            id_f32 = constp.tile([P, P], F32)
            make_identity(nc, id_f32)
            w_sb = constp.tile([P, 2 * CHUNKS * OUT], BF16)
            nc.sync.dma_start(w_sb[:, :], w_d.ap())
            ones_col = constp.tile([P, 1], F32)
            nc.vector.memset(ones_col[:, :], 1.0)
            ones_row = constp.tile([1, P], F32)
            nc.vector.memset(ones_row[:, :], 1.0)
            dT = constp.tile([OUT, OUT], F32)
            nc.sync.dma_start(dT[:, :], dT_d.ap())
            wbinT = constp.tile([OUT, OUT], F32)
            nc.sync.dma_start(wbinT[:, :], wbinT_d.ap())
            eb_row = constp.tile([1, OUT], F32)
            nc.sync.dma_start(eb_row[:, :], eb_d.ap())

            # persistent accumulators
            y_all = perp.tile([P, TILES * CHUNKS * OUT], F32)   # [128, 2560]
            red_all = perp.tile([P, TILES * CHUNKS], F32)       # [128, 256]

            # ---- main pass over x: stats + all-chunk projection ----
            for t in range(TILES):
                xb = xbp.tile([P, F], BF16, tag="xb")
                # SWDGE DMA with f32 -> bf16 cast in the datapath
                nc.gpsimd.dma_start(xb[:, :], x_d.ap()[t * P:(t + 1) * P, :])

                # per-chunk sum of |x| for this tile (fused abs+reduce)
                nc.vector.tensor_reduce(
                    red_all[:, t * CHUNKS:(t + 1) * CHUNKS],
                    xb[:, :].rearrange("p (c f) -> p c f", f=CHUNK),
                    axis=AX.X,
                    op=OP.add,
                    apply_absolute_value=True,
                )

                # transpose all 32 [128,128] blocks: x[b, f] -> xT[f, b]
                xt = xtp.tile([P, F], BF16, tag="xt")
                for g in range(8):
                    ps = pstr.tile([P, 4 * P], BF16, tag="ps_tr")
                    for j in range(4):
                        k = 4 * g + j
                        nc.tensor.transpose(
                            ps[:, j * P:(j + 1) * P],
                            xb[:, k * P:(k + 1) * P],
                            id_bf[:, :],
                        )
                    nc.scalar.copy(xt[:, g * 4 * P:(g + 1) * 4 * P], ps[:, :])

                # project every chunk: y[b, c, o] accumulated over 2 halves
                psy_t = psy.tile([P, CHUNKS * OUT], F32, tag="psy")
                for c in range(CHUNKS):
                    for h in range(2):
                        kh = 2 * c + h
                        nc.tensor.matmul(
                            psy_t[:, c * OUT:(c + 1) * OUT],
                            lhsT=xt[:, kh * P:(kh + 1) * P],
                            rhs=w_sb[:, h * CHUNKS * OUT + c * OUT:
                                     h * CHUNKS * OUT + c * OUT + OUT],
                            start=(c == 0 and h == 0),
                            stop=(c == CHUNKS - 1 and h == 1),
                        )
                nc.scalar.copy(
                    y_all[:, t * CHUNKS * OUT:(t + 1) * CHUNKS * OUT], psy_t[:, :]
                )

            # ---- finalize stats: reduce over tiles, then over partitions ----
            acts_p = tailp.tile([P, CHUNKS], F32, tag="acts_p")
            nc.vector.tensor_reduce(
                acts_p[:, :],
                red_all[:, :].rearrange("p (t c) -> p c t", c=CHUNKS),
                axis=AX.X,
                op=OP.add,
            )
            ps_acts = psm.tile([1, CHUNKS], F32, tag="tailps")
            nc.tensor.matmul(
                ps_acts[:, :], lhsT=ones_col[:, :], rhs=acts_p[:, :],
                start=True, stop=True,
            )
            cc_in_sb = tailp.tile([1, CHUNKS], F32, tag="cc_in_sb")
            nc.scalar.copy(cc_in_sb[:, :], ps_acts[:, :])

            cc_in = dramp.tile([1, CHUNKS], F32)
            cc_out = dramp.tile([1, CHUNKS], F32)
            nc.sync.dma_start(cc_in[:, :], cc_in_sb[:, :])
            nc.gpsimd.collective_compute(
                "AllReduce",
                OP.add,
                replica_groups=[list(range(N_CORES))],
                ins=[cc_in.opt()],
                outs=[cc_out.opt()],
            )
            S = tailp.tile([1, CHUNKS], F32, tag="S")
            nc.sync.dma_start(S[:, :], cc_out[:, :])

            # ---- top-4 threshold via 4x (max + mask-out); all on partition 0
            cur = tailp.tile([1, CHUNKS], F32, tag="cur")
            nc.vector.tensor_copy(cur[:, :], S[:, :])
            m1 = None
            mk = None
            for k in range(TOPK):
                mk = tailp.tile([1, 1], F32, tag=f"mk{k}")
                nc.vector.tensor_reduce(mk[:, :], cur[:, :], axis=AX.X, op=OP.max)
                if k == 0:
                    m1 = mk
                if k < TOPK - 1:
                    sel = tailp.tile([1, CHUNKS], F32, tag="sel")
                    nc.vector.tensor_scalar(
                        sel[:, :], cur[:, :], mk[:, :], None, op0=OP.is_ge
                    )
                    nc.vector.tensor_scalar(
                        sel[:, :], sel[:, :], BIG_NEG, None, op0=OP.mult
                    )
                    nc.vector.tensor_tensor(cur[:, :], cur[:, :], sel[:, :], op=OP.add)
            m4 = mk  # 4th largest

            mask16 = tailp.tile([1, CHUNKS], F32, tag="mask16")
            nc.vector.tensor_scalar(
                mask16[:, :], S[:, :], m4[:, :], None, op0=OP.is_ge
            )
            cond = tailp.tile([1, 1], F32, tag="cond")
            nc.vector.tensor_scalar(
                cond[:, :], m1[:, :], float(SUM_THRESH), None, op0=OP.is_gt
            )

            # ---- broadcast row -> all partitions via K=1 matmul ----
            # layout: [0:160] mask per (c,o), [160:170] expert_b, [170] cond
            BROW = CHUNKS * OUT + OUT + 1
            brow = tailp.tile([1, BROW], F32, tag="brow")
            for c in range(CHUNKS):
                nc.vector.tensor_scalar(
                    brow[:, c * OUT:(c + 1) * OUT],
                    ones_row[:, 0:OUT],
                    mask16[:, c:c + 1],
                    None,
                    op0=OP.mult,
                )
            nc.vector.tensor_copy(
                brow[:, CHUNKS * OUT:CHUNKS * OUT + OUT], eb_row[:, :]
            )
            nc.vector.tensor_copy(
                brow[:, CHUNKS * OUT + OUT:BROW], cond[:, :]
            )
            ps_b = psm.tile([P, BROW], F32, tag="tailps")
            nc.tensor.matmul(
                ps_b[:, :], lhsT=ones_row[:, :], rhs=brow[:, :],
                start=True, stop=True,
            )
            bc = tailp.tile([P, BROW], F32, tag="bc")
            nc.scalar.copy(bc[:, :], ps_b[:, :])
            maskwide = bc[:, 0:CHUNKS * OUT]
            eb_b = bc[:, CHUNKS * OUT:CHUNKS * OUT + OUT]
            cond_col = bc[0:OUT, CHUNKS * OUT + OUT:BROW]      # [10, 1]

            # ---- effective head weights: [11, 10] lhsT (last row = quant_b)
            waug = perp.tile([OUT + 1, OUT], F32)
            nc.sync.dma_start(waug[OUT:OUT + 1, :], qb_d.ap())
            wtmp = tailp.tile([OUT, OUT], F32, tag="wtmp")
            nc.vector.tensor_scalar(
                wtmp[:, :], dT[:, :], cond_col, None, op0=OP.mult
            )
            nc.vector.tensor_tensor(
                waug[0:OUT, :], wtmp[:, :], wbinT[:, :], op=OP.add
            )

            combT_aug = perp.tile([OUT + 1, P], F32)
            nc.vector.memset(combT_aug[:, :], 1.0)  # row 10 stays ones

            # ---- per-tile combine + quantized head ----
            for t in range(TILES):
                y_t = y_all[:, t * CHUNKS * OUT:(t + 1) * CHUNKS * OUT]
                tmp160 = tailp.tile([P, CHUNKS * OUT], F32, tag="tmp160")
                nc.vector.tensor_tensor(tmp160[:, :], y_t, maskwide, op=OP.mult)
                comb = tailp.tile([P, OUT], F32, tag="comb")
                nc.vector.tensor_reduce(
                    comb[:, :],
                    tmp160[:, :].rearrange("p (c o) -> p o c", o=OUT),
                    axis=AX.X,
                    op=OP.add,
                )
                nc.vector.tensor_tensor(comb[:, :], comb[:, :], eb_b, op=OP.add)

                ps_t1 = psm.tile([OUT, P], F32, tag="tailps")
                nc.tensor.transpose(ps_t1[:, :], comb[:, :], id_f32[:, :])
                nc.scalar.copy(combT_aug[0:OUT, :], ps_t1[:, :])

                ps_h = psm.tile([OUT, P], F32, tag="tailps")
                nc.tensor.matmul(
                    ps_h[:, :], lhsT=waug[:, :], rhs=combT_aug[:, :],
                    start=True, stop=True,
                )
                hT = tailp.tile([OUT, P], F32, tag="hT")
                nc.scalar.copy(hT[:, :], ps_h[:, :])

                ps_o = psm.tile([P, OUT], F32, tag="tailps")
                nc.tensor.transpose(ps_o[:, :], hT[:, :], id_f32[0:OUT, 0:OUT])
                outt = tailp.tile([P, OUT], F32, tag="outt")
                nc.scalar.copy(outt[:, :], ps_o[:, :])
                nc.sync.dma_start(out_d.ap()[t * P:(t + 1) * P, :], outt[:, :])

    nc.compile()
    return nc


def _get_nc():
    if "nc" not in _CACHE:
        _CACHE["nc"] = _build()
    return _CACHE["nc"]


def kernel(x, expert_w, expert_b, quant_w, quant_b):
    import ml_dtypes

    x = np.ascontiguousarray(np.asarray(x, dtype=np.float32))
    expert_w = np.asarray(expert_w, dtype=np.float32)
    expert_b = np.asarray(expert_b, dtype=np.float32)
    quant_w = np.asarray(quant_w, dtype=np.float32)
    quant_b = np.asarray(quant_b, dtype=np.float32)

    # host-side weight prep (tiny tensors only)
    # w_sb[p, h*160 + c*10 + o] = expert_w[c, o, h*128 + p]
    wr = expert_w.reshape(CHUNKS, OUT, 2, P)            # c, o, h, p
    w_sb = np.ascontiguousarray(
        wr.transpose(3, 2, 0, 1).reshape(P, 2 * CHUNKS * OUT)
    ).astype(ml_dtypes.bfloat16)
    qwT = quant_w.T.astype(np.float32)
    qmean = np.float32(np.mean(np.abs(quant_w)))
    wbinT = (np.sign(qwT) * qmean).astype(np.float32)
    dT = np.ascontiguousarray(qwT - wbinT)
    wbinT = np.ascontiguousarray(wbinT)

    nc = _get_nc()
    in_maps = []
    for i in range(N_CORES):
        in_maps.append({
            "x": np.ascontiguousarray(x[i * BC:(i + 1) * BC]),
            "w_sb": w_sb,
            "dT": dT,
            "wbinT": wbinT,
            "eb": np.ascontiguousarray(expert_b.reshape(1, OUT)),
            "qb": np.ascontiguousarray(quant_b.reshape(1, OUT)),
        })

    res = run_bass_kernel_spmd(nc, in_maps, core_ids=list(range(N_CORES)))
    out = np.concatenate(
        [np.asarray(res.results[i]["out"]) for i in range(N_CORES)], axis=0
    )
    return out.astype(np.float32)


# revision 2
# speedup vs baseline: 1.2932x; 1.2932x over previous
"""Trainium2 Bass kernel for nn_ChunkedQuantHead.

Computation (see reference):
  xc   = x.reshape(B, 16, 256)
  acts = mean(|xc|, axis=(0, 2))           # global per-chunk stat
  top4 = top_k(acts, 4)                    # global chunk routing
  routed = einsum('bkc,koc->bo', xc[:, top4], expert_w[top4]) + expert_b
  w_eff  = quant_w if max(acts) > 0.5 else sign(quant_w)*mean|quant_w|
  out    = routed @ w_eff.T + quant_b

Strategy (8 cores, data-parallel over batch):
  - Each core streams its 2048x4096 f32 shard from HBM ONCE (bf16 cast
    during the SWDGE DMA).  In the same pass it computes per-chunk
    |x| partial sums (DVE fused abs+reduce) AND projects ALL 16 chunks
    (y_all[b, c, o]) via PE (transpose blocks + bf16 matmuls) -- so no
    second pass over x is ever needed.
  - A 64-byte AllReduce combines the chunk stats; top-4 selection is
    done with masks (sum of top-4 needs no ordering), so there is no
    dynamic control flow: routed = sum_c mask_c * y_all[:, c, :].
  - The quantized head runs on-chip: w_eff = w_bin + cond*(qw - w_bin),
    folded into a tiny [11,10] matmul (bias row included).
"""

import numpy as np

import concourse.bacc as bacc
import concourse.tile as tile
import concourse.mybir as mybir
from concourse.bass_utils import run_bass_kernel_spmd
from concourse.masks import make_identity

F32 = mybir.dt.float32
BF16 = mybir.dt.bfloat16
AX = mybir.AxisListType
OP = mybir.AluOpType

N_CORES = 8
B, F = 16384, 4096
CHUNKS, CHUNK, OUT = 16, 256, 10
TOPK = 4
THRESH = 0.5
BC = B // N_CORES            # 2048 rows per core
P = 128
TILES = BC // P              # 16 tiles of 128 rows
KH = F // P                  # 32 128-feature half-chunk blocks
SUM_THRESH = THRESH * B * CHUNK  # compare sum(|x|) against this (scale folded)
BIG_NEG = -1.0e30

_CACHE = {}


def _build():
    nc = bacc.Bacc(
        "TRN2",
        target_bir_lowering=False,
        debug=False,
        num_devices=N_CORES,
    )

    x_d = nc.dram_tensor("x", [BC, F], F32, kind="ExternalInput")
    # expert weights pre-arranged host-side: w_sb[p, h*160 + c*10 + o]
    #   = expert_w[c, o, h*128 + p]   (bf16)
    w_d = nc.dram_tensor("w_sb", [P, 2 * CHUNKS * OUT], BF16, kind="ExternalInput")
    # head weights, transposed layouts prepared host-side
    dT_d = nc.dram_tensor("dT", [OUT, OUT], F32, kind="ExternalInput")       # qw.T - w_bin.T
    wbinT_d = nc.dram_tensor("wbinT", [OUT, OUT], F32, kind="ExternalInput")  # sign(qw.T)*mean|qw|
    eb_d = nc.dram_tensor("eb", [1, OUT], F32, kind="ExternalInput")
    qb_d = nc.dram_tensor("qb", [1, OUT], F32, kind="ExternalInput")
    out_d = nc.dram_tensor("out", [BC, OUT], F32, kind="ExternalOutput")

    AG_SPLIT = 12  # tiles 0..11 go in the early AllGather, 12..15 in the late one

    with tile.TileContext(nc) as tc:
        with (
            tc.tile_pool(name="const", bufs=1) as constp,
            tc.tile_pool(name="persist", bufs=1) as perp,
            tc.tile_pool(name="xb", bufs=3) as xbp,
            tc.tile_pool(name="xt", bufs=2) as xtp,
            tc.tile_pool(name="tail", bufs=3) as tailp,
            tc.tile_pool(name="ps_misc", bufs=2, space="PSUM") as psm,
            tc.tile_pool(name="dram", bufs=1, space="DRAM") as dramp,
        ):
            # ---- constants ----
            id_bf = constp.tile([P, P], BF16)
            make_identity(nc, id_bf)
            id_f32 = constp.tile([P, P], F32)
            make_identity(nc, id_f32)
            w_sb = constp.tile([P, 2 * CHUNKS * OUT], BF16)
            nc.sync.dma_start(w_sb[:, :], w_d.ap())
            ones_col = constp.tile([P, 1], F32)
            nc.vector.memset(ones_col[:, :], 1.0)
            ones_row = constp.tile([1, P], F32)
            nc.vector.memset(ones_row[:, :], 1.0)
            dT = constp.tile([OUT, OUT], F32)
            nc.sync.dma_start(dT[:, :], dT_d.ap())
            wbinT = constp.tile([OUT, OUT], F32)
            nc.sync.dma_start(wbinT[:, :], wbinT_d.ap())
            eb_row = constp.tile([1, OUT], F32)
            nc.sync.dma_start(eb_row[:, :], eb_d.ap())
            qb_row = constp.tile([1, OUT], F32)
            nc.sync.dma_start(qb_row[:, :], qb_d.ap())

            # persistent accumulators
            y_all = perp.tile([P, TILES * CHUNKS * OUT], F32)   # [128, 2560]
            red_all = perp.tile([P, TILES * CHUNKS], F32)       # [128, 256]

            # DRAM bounce buffers for the two AllGathers
            cc1_in = dramp.tile([1, CHUNKS], F32)
            cc1_out = dramp.tile([N_CORES, CHUNKS], F32)
            cc2_in = dramp.tile([1, CHUNKS], F32)
            cc2_out = dramp.tile([N_CORES, CHUNKS], F32)

            def emit_partial_allgather(t_lo, t_hi, cc_in, cc_out, idx):
                """Partition-reduce stats for tiles [t_lo, t_hi) and AllGather."""
                nt = t_hi - t_lo
                acts_p = tailp.tile([P, CHUNKS], F32, tag=f"acts_p{idx}")
                nc.vector.tensor_reduce(
                    acts_p[:, :],
                    red_all[:, t_lo * CHUNKS:t_hi * CHUNKS].rearrange(
                        "p (t c) -> p c t", c=CHUNKS
                    ),
                    axis=AX.X,
                    op=OP.add,
                )
                ps_a = psm.tile([1, CHUNKS], F32, tag="psmisc")
                nc.tensor.matmul(
                    ps_a[:, :], lhsT=ones_col[:, :], rhs=acts_p[:, :],
                    start=True, stop=True,
                )
                cc_sb = tailp.tile([1, CHUNKS], F32, tag=f"cc_sb{idx}")
                nc.scalar.copy(cc_sb[:, :], ps_a[:, :])
                nc.sync.dma_start(cc_in[:, :], cc_sb[:, :])
                nc.gpsimd.collective_compute(
                    "AllGather",
                    OP.bypass,
                    replica_groups=[list(range(N_CORES))],
                    ins=[cc_in.opt()],
                    outs=[cc_out.opt()],
                )

            # ---- main pass over x: stats + all-chunk projection ----
            with (
                tc.tile_pool(name="ps_tr", bufs=2, space="PSUM") as pstr,
                tc.tile_pool(name="ps_y", bufs=2, space="PSUM") as psy,
            ):
                for t in range(TILES):
                    xb = xbp.tile([P, F], BF16, tag="xb")
                    # SWDGE DMA with f32 -> bf16 cast in the datapath
                    nc.gpsimd.dma_start(xb[:, :], x_d.ap()[t * P:(t + 1) * P, :])

                    # per-chunk sum of |x| for this tile (fused abs+reduce)
                    nc.vector.tensor_reduce(
                        red_all[:, t * CHUNKS:(t + 1) * CHUNKS],
                        xb[:, :].rearrange("p (c f) -> p c f", f=CHUNK),
                        axis=AX.X,
                        op=OP.add,
                        apply_absolute_value=True,
                    )

                    # transpose all 32 [128,128] blocks: x[b, f] -> xT[f, b]
                    xt = xtp.tile([P, F], BF16, tag="xt")
                    for g in range(2):
                        ps = pstr.tile([P, 16 * P], BF16, tag="ps_tr")
                        for j in range(16):
                            k = 16 * g + j
                            nc.tensor.transpose(
                                ps[:, j * P:(j + 1) * P],
                                xb[:, k * P:(k + 1) * P],
                                id_bf[:, :],
                            )
                        nc.scalar.copy(
                            xt[:, g * 16 * P:(g + 1) * 16 * P], ps[:, :]
                        )

                    # project every chunk: y[b, c, o] accumulated over 2 halves
                    psy_t = psy.tile([P, CHUNKS * OUT], F32, tag="psy")
                    for c in range(CHUNKS):
                        for h in range(2):
                            kh = 2 * c + h
                            nc.tensor.matmul(
                                psy_t[:, c * OUT:(c + 1) * OUT],
                                lhsT=xt[:, kh * P:(kh + 1) * P],
                                rhs=w_sb[:, h * CHUNKS * OUT + c * OUT:
                                         h * CHUNKS * OUT + c * OUT + OUT],
                                start=(c == 0 and h == 0),
                                stop=(c == CHUNKS - 1 and h == 1),
                            )
                    nc.scalar.copy(
                        y_all[:, t * CHUNKS * OUT:(t + 1) * CHUNKS * OUT],
                        psy_t[:, :],
                    )

                    if t == AG_SPLIT - 1:
                        # early AllGather covering tiles 0..AG_SPLIT-1 --
                        # overlaps with the rest of the main loop
                        emit_partial_allgather(0, AG_SPLIT, cc1_in, cc1_out, 1)

                # late AllGather for the remaining tiles
                emit_partial_allgather(AG_SPLIT, TILES, cc2_in, cc2_out, 2)

            # ---- combine the two gathers: S = sum over 16 partials ----
            Sg = tailp.tile([2 * N_CORES, CHUNKS], F32, tag="Sg")
            nc.sync.dma_start(Sg[0:N_CORES, :], cc1_out[:, :])
            nc.sync.dma_start(Sg[N_CORES:2 * N_CORES, :], cc2_out[:, :])
            ps_s = psm.tile([1, CHUNKS], F32, tag="psmisc")
            nc.tensor.matmul(
                ps_s[:, :], lhsT=ones_col[0:2 * N_CORES, :], rhs=Sg[:, :],
                start=True, stop=True,
            )
            S = tailp.tile([1, CHUNKS], F32, tag="S")
            nc.scalar.copy(S[:, :], ps_s[:, :])

            # ---- top-4 threshold via 4x (max + mask-out); all on partition 0
            cur = tailp.tile([1, CHUNKS], F32, tag="cur")
            nc.vector.tensor_copy(cur[:, :], S[:, :])
            m1 = None
            mk = None
            for k in range(TOPK):
                mk = tailp.tile([1, 1], F32, tag=f"mk{k}")
                nc.vector.tensor_reduce(mk[:, :], cur[:, :], axis=AX.X, op=OP.max)
                if k == 0:
                    m1 = mk
                if k < TOPK - 1:
                    sel = tailp.tile([1, CHUNKS], F32, tag="sel")
                    # sel = (cur >= mk) * BIG_NEG  in one fused op
                    nc.vector.tensor_scalar(
                        sel[:, :], cur[:, :], mk[:, :], BIG_NEG,
                        op0=OP.is_ge, op1=OP.mult,
                    )
                    nc.vector.tensor_tensor(cur[:, :], cur[:, :], sel[:, :], op=OP.add)
            m4 = mk  # 4th largest

            mask16 = tailp.tile([1, CHUNKS], F32, tag="mask16")
            nc.vector.tensor_scalar(
                mask16[:, :], S[:, :], m4[:, :], None, op0=OP.is_ge
            )
            cond = tailp.tile([1, 1], F32, tag="cond")
            nc.vector.tensor_scalar(
                cond[:, :], m1[:, :], float(SUM_THRESH), None, op0=OP.is_gt
            )

            # ---- broadcast row -> all partitions via K=1 matmul ----
            # layout: [0:160] mask per (c,o), [160:170] expert_b,
            #         [170:180] quant_b, [180] cond
            BROW = CHUNKS * OUT + 2 * OUT + 1
            brow = tailp.tile([1, BROW], F32, tag="brow")
            for c in range(CHUNKS):
                nc.vector.tensor_scalar(
                    brow[:, c * OUT:(c + 1) * OUT],
                    ones_row[:, 0:OUT],
                    mask16[:, c:c + 1],
                    None,
                    op0=OP.mult,
                )
            nc.vector.tensor_copy(
                brow[:, CHUNKS * OUT:CHUNKS * OUT + OUT], eb_row[:, :]
            )
            nc.vector.tensor_copy(
                brow[:, CHUNKS * OUT + OUT:CHUNKS * OUT + 2 * OUT], qb_row[:, :]
            )
            nc.vector.tensor_copy(
                brow[:, CHUNKS * OUT + 2 * OUT:BROW], cond[:, :]
            )
            ps_b = psm.tile([P, BROW], F32, tag="psmisc")
            nc.tensor.matmul(
                ps_b[:, :], lhsT=ones_row[:, :], rhs=brow[:, :],
                start=True, stop=True,
            )
            bc = tailp.tile([P, BROW], F32, tag="bc")
            nc.scalar.copy(bc[:, :], ps_b[:, :])
            maskwide = bc[:, 0:CHUNKS * OUT]
            eb_b = bc[:, CHUNKS * OUT:CHUNKS * OUT + OUT]
            qb_b = bc[:, CHUNKS * OUT + OUT:CHUNKS * OUT + 2 * OUT]
            cond_col = bc[0:OUT, CHUNKS * OUT + 2 * OUT:BROW]      # [10, 1]

            # ---- effective head weights [10, 10]: w_eff^T as lhsT ----
            weff = perp.tile([OUT, OUT], F32)
            wtmp = tailp.tile([OUT, OUT], F32, tag="wtmp")
            nc.vector.tensor_scalar(
                wtmp[:, :], dT[:, :], cond_col, None, op0=OP.mult
            )
            nc.vector.tensor_tensor(weff[:, :], wtmp[:, :], wbinT[:, :], op=OP.add)

            # ---- per-tile combine + quantized head (pipelined) ----
            with tc.tile_pool(name="ps_tail", bufs=2, space="PSUM") as pstail:
                for t in range(TILES):
                    y_t = y_all[:, t * CHUNKS * OUT:(t + 1) * CHUNKS * OUT]
                    tmp160 = tailp.tile([P, CHUNKS * OUT], F32, tag="tmp160")
                    nc.vector.tensor_tensor(
                        tmp160[:, :], y_t, maskwide, op=OP.mult
                    )
                    comb = tailp.tile([P, OUT], F32, tag="comb")
                    nc.vector.tensor_reduce(
                        comb[:, :],
                        tmp160[:, :].rearrange("p (c o) -> p o c", o=OUT),
                        axis=AX.X,
                        op=OP.add,
                    )
                    nc.vector.tensor_tensor(comb[:, :], comb[:, :], eb_b, op=OP.add)

                    ps_t1 = pstail.tile([OUT, P], F32, tag="ps1")
                    nc.tensor.transpose(ps_t1[:, :], comb[:, :], id_f32[:, :])
                    combT = tailp.tile([OUT, P], F32, tag="combT")
                    nc.scalar.copy(combT[:, :], ps_t1[:, :])

                    ps_h = pstail.tile([OUT, P], F32, tag="ps2")
                    nc.tensor.matmul(
                        ps_h[:, :], lhsT=weff[:, :], rhs=combT[:, :],
                        start=True, stop=True,
                    )
                    hT = tailp.tile([OUT, P], F32, tag="hT")
                    nc.scalar.copy(hT[:, :], ps_h[:, :])

                    ps_o = pstail.tile([P, OUT], F32, tag="ps3")
                    nc.tensor.transpose(
                        ps_o[:, :], hT[:, :], id_f32[0:OUT, 0:OUT]
                    )
                    outt = tailp.tile([P, OUT], F32, tag="outt")
                    nc.scalar.copy(outt[:, :], ps_o[:, :])
                    nc.vector.tensor_tensor(
                        outt[:, :], outt[:, :], qb_b, op=OP.add
                    )
                    nc.sync.dma_start(
                        out_d.ap()[t * P:(t + 1) * P, :], outt[:, :]
                    )

    nc.compile()
    return nc


def _get_nc():
    if "nc" not in _CACHE:
        _CACHE["nc"] = _build()
    return _CACHE["nc"]


def kernel(x, expert_w, expert_b, quant_w, quant_b):
    import ml_dtypes

    x = np.ascontiguousarray(np.asarray(x, dtype=np.float32))
    expert_w = np.asarray(expert_w, dtype=np.float32)
    expert_b = np.asarray(expert_b, dtype=np.float32)
    quant_w = np.asarray(quant_w, dtype=np.float32)
    quant_b = np.asarray(quant_b, dtype=np.float32)

    # host-side weight prep (tiny tensors only)
    # w_sb[p, h*160 + c*10 + o] = expert_w[c, o, h*128 + p]
    wr = expert_w.reshape(CHUNKS, OUT, 2, P)            # c, o, h, p
    w_sb = np.ascontiguousarray(
        wr.transpose(3, 2, 0, 1).reshape(P, 2 * CHUNKS * OUT)
    ).astype(ml_dtypes.bfloat16)
    qwT = quant_w.T.astype(np.float32)
    qmean = np.float32(np.mean(np.abs(quant_w)))
    wbinT = (np.sign(qwT) * qmean).astype(np.float32)
    dT = np.ascontiguousarray(qwT - wbinT)
    wbinT = np.ascontiguousarray(wbinT)

    nc = _get_nc()
    in_maps = []
    for i in range(N_CORES):
        in_maps.append({
            "x": np.ascontiguousarray(x[i * BC:(i + 1) * BC]),
            "w_sb": w_sb,
            "dT": dT,
            "wbinT": wbinT,
            "eb": np.ascontiguousarray(expert_b.reshape(1, OUT)),
            "qb": np.ascontiguousarray(quant_b.reshape(1, OUT)),
        })

    res = run_bass_kernel_spmd(nc, in_maps, core_ids=list(range(N_CORES)))
    out = np.concatenate(
        [np.asarray(res.results[i]["out"]) for i in range(N_CORES)], axis=0
    )
    return out.astype(np.float32)


# revision 5
# speedup vs baseline: 1.4187x; 1.0971x over previous
"""Trainium2 Bass kernel for nn_ChunkedQuantHead.

Computation (see reference):
  xc   = x.reshape(B, 16, 256)
  acts = mean(|xc|, axis=(0, 2))           # global per-chunk stat
  top4 = top_k(acts, 4)                    # global chunk routing
  routed = einsum('bkc,koc->bo', xc[:, top4], expert_w[top4]) + expert_b
  w_eff  = quant_w if max(acts) > 0.5 else sign(quant_w)*mean|quant_w|
  out    = routed @ w_eff.T + quant_b

Strategy (8 cores, data-parallel over batch):
  - Each core streams its 2048x4096 f32 shard from HBM ONCE (bf16 cast
    during the SWDGE DMA).  In the same pass it computes per-chunk
    |x| partial sums (DVE fused abs+reduce) AND projects ALL 16 chunks
    (y_all[b, c, o]) via PE (transpose blocks + bf16 matmuls) -- so no
    second pass over x is ever needed.
  - A 64-byte AllReduce combines the chunk stats; top-4 selection is
    done with masks (sum of top-4 needs no ordering), so there is no
    dynamic control flow: routed = sum_c mask_c * y_all[:, c, :].
  - The quantized head runs on-chip: w_eff = w_bin + cond*(qw - w_bin),
    folded into a tiny [11,10] matmul (bias row included).
"""

import numpy as np

import concourse.bacc as bacc
import concourse.tile as tile
import concourse.mybir as mybir
from concourse.bass_utils import run_bass_kernel_spmd
from concourse.masks import make_identity

F32 = mybir.dt.float32
BF16 = mybir.dt.bfloat16
AX = mybir.AxisListType
OP = mybir.AluOpType

N_CORES = 8
B, F = 16384, 4096
CHUNKS, CHUNK, OUT = 16, 256, 10
TOPK = 4
THRESH = 0.5
BC = B // N_CORES            # 2048 rows per core
P = 128
TILES = BC // P              # 16 tiles of 128 rows
KH = F // P                  # 32 128-feature half-chunk blocks
SUM_THRESH = THRESH * B * CHUNK  # compare sum(|x|) against this (scale folded)
BIG_NEG = -1.0e30

_CACHE = {}


def _build():
    nc = bacc.Bacc(
        "TRN2",
        target_bir_lowering=False,
        debug=False,
        num_devices=N_CORES,
    )

    x_d = nc.dram_tensor("x", [BC, F], F32, kind="ExternalInput")
    # expert weights pre-arranged host-side: w_sb[p, h*160 + c*10 + o]
    #   = expert_w[c, o, h*128 + p]   (bf16)
    w_d = nc.dram_tensor("w_sb", [P, 2 * CHUNKS * OUT], BF16, kind="ExternalInput")
    # head weights, transposed layouts prepared host-side
    dT_d = nc.dram_tensor("dT", [OUT, OUT], F32, kind="ExternalInput")       # qw.T - w_bin.T
    wbinT_d = nc.dram_tensor("wbinT", [OUT, OUT], F32, kind="ExternalInput")  # sign(qw.T)*mean|qw|
    eb_d = nc.dram_tensor("eb", [1, OUT], F32, kind="ExternalInput")
    qb_d = nc.dram_tensor("qb", [1, OUT], F32, kind="ExternalInput")
    out_d = nc.dram_tensor("out", [BC, OUT], F32, kind="ExternalOutput")

    AG_SPLIT = 10  # tiles 0..9 go in the early AllGather, 10..15 in the late one

    with tile.TileContext(nc) as tc:
        with (
            tc.tile_pool(name="const", bufs=1) as constp,
            tc.tile_pool(name="persist", bufs=1) as perp,
            tc.tile_pool(name="xb", bufs=3) as xbp,
            tc.tile_pool(name="xt", bufs=2) as xtp,
            tc.tile_pool(name="tail", bufs=3) as tailp,
            tc.tile_pool(name="ps_misc", bufs=2, space="PSUM") as psm,
            tc.tile_pool(name="dram", bufs=1, space="DRAM") as dramp,
        ):
            # ---- constants ----
            id_bf = constp.tile([P, P], BF16)
            make_identity(nc, id_bf)
            id_f32 = constp.tile([P, P], F32)
            make_identity(nc, id_f32)
            w_sb = constp.tile([P, 2 * CHUNKS * OUT], BF16)
            nc.sync.dma_start(w_sb[:, :], w_d.ap())
            ones_col = constp.tile([P, 1], F32)
            nc.vector.memset(ones_col[:, :], 1.0)
            ones_row = constp.tile([1, P], F32)
            nc.vector.memset(ones_row[:, :], 1.0)
            dT = constp.tile([OUT, OUT], F32)
            nc.sync.dma_start(dT[:, :], dT_d.ap())
            wbinT = constp.tile([OUT, OUT], F32)
            nc.sync.dma_start(wbinT[:, :], wbinT_d.ap())
            eb_row = constp.tile([1, OUT], F32)
            nc.sync.dma_start(eb_row[:, :], eb_d.ap())
            qb_row = constp.tile([1, OUT], F32)
            nc.sync.dma_start(qb_row[:, :], qb_d.ap())

            # persistent accumulators
            y_all = perp.tile([P, TILES * CHUNKS * OUT], F32)   # [128, 2560]
            red_all = perp.tile([P, TILES * CHUNKS], F32)       # [128, 256]

            # DRAM bounce buffers for the two AllGathers
            cc1_in = dramp.tile([1, CHUNKS], F32)
            cc1_out = dramp.tile([N_CORES, CHUNKS], F32)
            cc2_in = dramp.tile([1, CHUNKS], F32)
            cc2_out = dramp.tile([N_CORES, CHUNKS], F32)

            def emit_partial_allgather(t_lo, t_hi, cc_in, cc_out, idx):
                """Partition-reduce stats for tiles [t_lo, t_hi) and AllGather."""
                nt = t_hi - t_lo
                acts_p = tailp.tile([P, CHUNKS], F32, tag=f"acts_p{idx}")
                nc.vector.tensor_reduce(
                    acts_p[:, :],
                    red_all[:, t_lo * CHUNKS:t_hi * CHUNKS].rearrange(
                        "p (t c) -> p c t", c=CHUNKS
                    ),
                    axis=AX.X,
                    op=OP.add,
                )
                ps_a = psm.tile([1, CHUNKS], F32, tag="psmisc")
                nc.tensor.matmul(
                    ps_a[:, :], lhsT=ones_col[:, :], rhs=acts_p[:, :],
                    start=True, stop=True,
                )
                cc_sb = tailp.tile([1, CHUNKS], F32, tag=f"cc_sb{idx}")
                nc.scalar.copy(cc_sb[:, :], ps_a[:, :])
                nc.sync.dma_start(cc_in[:, :], cc_sb[:, :])
                nc.gpsimd.collective_compute(
                    "AllGather",
                    OP.bypass,
                    replica_groups=[list(range(N_CORES))],
                    ins=[cc_in.opt()],
                    outs=[cc_out.opt()],
                )

            # ---- main pass over x: stats + all-chunk projection ----
            with (
                tc.tile_pool(name="ps_tr", bufs=2, space="PSUM") as pstr,
                tc.tile_pool(name="ps_y", bufs=2, space="PSUM") as psy,
            ):
                for t in range(TILES):
                    xb = xbp.tile([P, F], BF16, tag="xb")
                    # SWDGE DMA with f32 -> bf16 cast in the datapath
                    nc.gpsimd.dma_start(xb[:, :], x_d.ap()[t * P:(t + 1) * P, :])

                    # per-chunk sum of |x| for this tile (fused abs+reduce)
                    nc.vector.tensor_reduce(
                        red_all[:, t * CHUNKS:(t + 1) * CHUNKS],
                        xb[:, :].rearrange("p (c f) -> p c f", f=CHUNK),
                        axis=AX.X,
                        op=OP.add,
                        apply_absolute_value=True,
                    )

                    # transpose all 32 [128,128] blocks: x[b, f] -> xT[f, b]
                    xt = xtp.tile([P, F], BF16, tag="xt")
                    for g in range(2):
                        ps = pstr.tile([P, 16 * P], BF16, tag="ps_tr")
                        for j in range(16):
                            k = 16 * g + j
                            nc.tensor.transpose(
                                ps[:, j * P:(j + 1) * P],
                                xb[:, k * P:(k + 1) * P],
                                id_bf[:, :],
                            )
                        nc.scalar.copy(
                            xt[:, g * 16 * P:(g + 1) * 16 * P], ps[:, :]
                        )

                    # project every chunk: y[b, c, o] accumulated over 2 halves
                    psy_t = psy.tile([P, CHUNKS * OUT], F32, tag="psy")
                    for c in range(CHUNKS):
                        for h in range(2):
                            kh = 2 * c + h
                            nc.tensor.matmul(
                                psy_t[:, c * OUT:(c + 1) * OUT],
                                lhsT=xt[:, kh * P:(kh + 1) * P],
                                rhs=w_sb[:, h * CHUNKS * OUT + c * OUT:
                                         h * CHUNKS * OUT + c * OUT + OUT],
                                start=(c == 0 and h == 0),
                                stop=(c == CHUNKS - 1 and h == 1),
                            )
                    nc.scalar.copy(
                        y_all[:, t * CHUNKS * OUT:(t + 1) * CHUNKS * OUT],
                        psy_t[:, :],
                    )

                    if t == AG_SPLIT - 1:
                        # early AllGather covering tiles 0..AG_SPLIT-1 --
                        # overlaps with the rest of the main loop
                        emit_partial_allgather(0, AG_SPLIT, cc1_in, cc1_out, 1)

                # late AllGather for the remaining tiles
                emit_partial_allgather(AG_SPLIT, TILES, cc2_in, cc2_out, 2)

            # ---- combine the two gathers: S = sum over 16 partials ----
            Sg = tailp.tile([2 * N_CORES, CHUNKS], F32, tag="Sg")
            nc.sync.dma_start(Sg[0:N_CORES, :], cc1_out[:, :])
            nc.sync.dma_start(Sg[N_CORES:2 * N_CORES, :], cc2_out[:, :])
            ps_s = psm.tile([1, CHUNKS], F32, tag="psmisc")
            nc.tensor.matmul(
                ps_s[:, :], lhsT=ones_col[0:2 * N_CORES, :], rhs=Sg[:, :],
                start=True, stop=True,
            )
            S = tailp.tile([1, CHUNKS], F32, tag="S")
            nc.scalar.copy(S[:, :], ps_s[:, :])

            # ---- top-4 threshold via 4x (max + mask-out); all on partition 0
            cur = tailp.tile([1, CHUNKS], F32, tag="cur")
            nc.vector.tensor_copy(cur[:, :], S[:, :])
            m1 = None
            mk = None
            for k in range(TOPK):
                mk = tailp.tile([1, 1], F32, tag=f"mk{k}")
                nc.vector.tensor_reduce(mk[:, :], cur[:, :], axis=AX.X, op=OP.max)
                if k == 0:
                    m1 = mk
                if k < TOPK - 1:
                    sel = tailp.tile([1, CHUNKS], F32, tag="sel")
                    # sel = (cur >= mk) * BIG_NEG  in one fused op
                    nc.vector.tensor_scalar(
                        sel[:, :], cur[:, :], mk[:, :], BIG_NEG,
                        op0=OP.is_ge, op1=OP.mult,
                    )
                    nc.vector.tensor_tensor(cur[:, :], cur[:, :], sel[:, :], op=OP.add)
            m4 = mk  # 4th largest

            mask16 = tailp.tile([1, CHUNKS], F32, tag="mask16")
            nc.vector.tensor_scalar(
                mask16[:, :], S[:, :], m4[:, :], None, op0=OP.is_ge
            )
            cond = tailp.tile([1, 1], F32, tag="cond")
            nc.vector.tensor_scalar(
                cond[:, :], m1[:, :], float(SUM_THRESH), None, op0=OP.is_gt
            )

            # ---- broadcast row -> all partitions via K=1 matmul ----
            # layout: [0:160] mask per (c,o), [160:170] expert_b,
            #         [170:180] quant_b, [180] cond
            BROW = CHUNKS * OUT + 2 * OUT + 1
            brow = tailp.tile([1, BROW], F32, tag="brow")
            for c in range(CHUNKS):
                nc.vector.tensor_scalar(
                    brow[:, c * OUT:(c + 1) * OUT],
                    ones_row[:, 0:OUT],
                    mask16[:, c:c + 1],
                    None,
                    op0=OP.mult,
                )
            nc.vector.tensor_copy(
                brow[:, CHUNKS * OUT:CHUNKS * OUT + OUT], eb_row[:, :]
            )
            nc.vector.tensor_copy(
                brow[:, CHUNKS * OUT + OUT:CHUNKS * OUT + 2 * OUT], qb_row[:, :]
            )
            nc.vector.tensor_copy(
                brow[:, CHUNKS * OUT + 2 * OUT:BROW], cond[:, :]
            )
            ps_b = psm.tile([P, BROW], F32, tag="psmisc")
            nc.tensor.matmul(
                ps_b[:, :], lhsT=ones_row[:, :], rhs=brow[:, :],
                start=True, stop=True,
            )
            bc = tailp.tile([P, BROW], F32, tag="bc")
            nc.scalar.copy(bc[:, :], ps_b[:, :])
            maskwide = bc[:, 0:CHUNKS * OUT]
            eb_b = bc[:, CHUNKS * OUT:CHUNKS * OUT + OUT]
            qb_b = bc[:, CHUNKS * OUT + OUT:CHUNKS * OUT + 2 * OUT]
            cond_col = bc[0:OUT, CHUNKS * OUT + 2 * OUT:BROW]      # [10, 1]

            # ---- effective head weights [10, 10]: w_eff^T as lhsT ----
            weff = perp.tile([OUT, OUT], F32)
            wtmp = tailp.tile([OUT, OUT], F32, tag="wtmp")
            nc.vector.tensor_scalar(
                wtmp[:, :], dT[:, :], cond_col, None, op0=OP.mult
            )
            nc.vector.tensor_tensor(weff[:, :], wtmp[:, :], wbinT[:, :], op=OP.add)

            # block-diagonal 4x copy of w_eff^T (via SBUF->SBUF DMA: writes at
            # arbitrary partition offsets); GT = 4 tiles handled per group
            GT = 4
            W4 = GT * OUT
            weff4 = perp.tile([W4, W4], F32)
            nc.vector.memset(weff4[:, :], 0.0)
            for j in range(GT):
                nc.sync.dma_start(
                    weff4[j * OUT:(j + 1) * OUT, j * OUT:(j + 1) * OUT],
                    weff[:, :],
                )

            # replicate the mask row 4x -> bc4 [128, 640] so 4 tiles of
            # y_all can be masked in a single DVE op
            bc4 = perp.tile([P, GT * CHUNKS * OUT], F32)
            ps_b4 = psm.tile([P, CHUNKS * OUT], F32, tag="psmisc")
            nc.tensor.matmul(
                ps_b4[:, :], lhsT=ones_row[:, :], rhs=brow[:, 0:CHUNKS * OUT],
                start=True, stop=True,
            )
            for j in range(GT):
                nc.scalar.copy(
                    bc4[:, j * CHUNKS * OUT:(j + 1) * CHUNKS * OUT], ps_b4[:, :]
                )

            # ---- grouped combine + quantized head: 4 tiles per iteration ----
            with tc.tile_pool(name="ps_tail", bufs=2, space="PSUM") as pstail:
                for g in range(TILES // GT):
                    t0 = g * GT
                    y_g = y_all[:, t0 * CHUNKS * OUT:(t0 + GT) * CHUNKS * OUT]
                    tmp4 = tailp.tile([P, GT * CHUNKS * OUT], F32, tag="tmp4")
                    nc.vector.tensor_tensor(tmp4[:, :], y_g, bc4[:, :], op=OP.mult)
                    # reduce over c: [p, (j c o)] -> [p, (j o)]
                    comb4 = tailp.tile([P, W4], F32, tag="comb4")
                    nc.vector.tensor_reduce(
                        comb4[:, :],
                        tmp4[:, :].rearrange("p (j c o) -> p j o c", j=GT, o=OUT),
                        axis=AX.X,
                        op=OP.add,
                    )
                    for j in range(GT):
                        nc.vector.tensor_tensor(
                            comb4[:, j * OUT:(j + 1) * OUT],
                            comb4[:, j * OUT:(j + 1) * OUT],
                            eb_b, op=OP.add,
                        )

                    ps_t1 = pstail.tile([W4, P], F32, tag="ps1")
                    nc.tensor.transpose(ps_t1[:, :], comb4[:, :], id_f32[:, :])
                    combT = tailp.tile([W4, P], F32, tag="combT")
                    nc.scalar.copy(combT[:, :], ps_t1[:, :])

                    ps_h = pstail.tile([W4, P], F32, tag="ps2")
                    nc.tensor.matmul(
                        ps_h[:, :], lhsT=weff4[:, :], rhs=combT[:, :],
                        start=True, stop=True,
                    )
                    hT = tailp.tile([W4, P], F32, tag="hT")
                    nc.scalar.copy(hT[:, :], ps_h[:, :])

                    ps_o = pstail.tile([P, W4], F32, tag="ps3")
                    nc.tensor.transpose(
                        ps_o[:, :], hT[:, :], id_f32[0:W4, 0:W4]
                    )
                    outt = tailp.tile([P, W4], F32, tag="outt")
                    nc.scalar.copy(outt[:, :], ps_o[:, :])
                    for j in range(GT):
                        nc.vector.tensor_tensor(
                            outt[:, j * OUT:(j + 1) * OUT],
                            outt[:, j * OUT:(j + 1) * OUT],
                            qb_b, op=OP.add,
                        )
                    # one DMA for 4 tiles: SBUF [p, (j o)] -> DRAM rows
                    # (t0+j)*128 + p
                    nc.sync.dma_start(
                        out_d.ap()[t0 * P:(t0 + GT) * P, :].rearrange(
                            "(j p) o -> p j o", p=P
                        ),
                        outt[:, :].rearrange("p (j o) -> p j o", o=OUT),
                    )

    nc.compile()
    return nc


def _get_nc():
    if "nc" not in _CACHE:
        _CACHE["nc"] = _build()
    return _CACHE["nc"]


def kernel(x, expert_w, expert_b, quant_w, quant_b):
    import ml_dtypes

    x = np.ascontiguousarray(np.asarray(x, dtype=np.float32))
    expert_w = np.asarray(expert_w, dtype=np.float32)
    expert_b = np.asarray(expert_b, dtype=np.float32)
    quant_w = np.asarray(quant_w, dtype=np.float32)
    quant_b = np.asarray(quant_b, dtype=np.float32)

    # host-side weight prep (tiny tensors only)
    # w_sb[p, h*160 + c*10 + o] = expert_w[c, o, h*128 + p]
    wr = expert_w.reshape(CHUNKS, OUT, 2, P)            # c, o, h, p
    w_sb = np.ascontiguousarray(
        wr.transpose(3, 2, 0, 1).reshape(P, 2 * CHUNKS * OUT)
    ).astype(ml_dtypes.bfloat16)
    qwT = quant_w.T.astype(np.float32)
    qmean = np.float32(np.mean(np.abs(quant_w)))
    wbinT = (np.sign(qwT) * qmean).astype(np.float32)
    dT = np.ascontiguousarray(qwT - wbinT)
    wbinT = np.ascontiguousarray(wbinT)

    nc = _get_nc()
    in_maps = []
    for i in range(N_CORES):
        in_maps.append({
            "x": np.ascontiguousarray(x[i * BC:(i + 1) * BC]),
            "w_sb": w_sb,
            "dT": dT,
            "wbinT": wbinT,
            "eb": np.ascontiguousarray(expert_b.reshape(1, OUT)),
            "qb": np.ascontiguousarray(quant_b.reshape(1, OUT)),
        })

    res = run_bass_kernel_spmd(nc, in_maps, core_ids=list(range(N_CORES)))
    out = np.concatenate(
        [np.asarray(res.results[i]["out"]) for i in range(N_CORES)], axis=0
    )
    return out.astype(np.float32)
